# revision 25
# baseline (speedup 1.0000x reference)
"""Trainium2 Bass kernel for nn_DenseNet3D_89730456748628.

Reference structure (after dead-code elimination):
  - The reference builds seq (B=64, T=512, 192) and runs two BiGRUs with
    sequence axis = B (64 steps) and batch axis = T (512).  The decoder
    consumes only dec_h_all[:64] and y2 is discarded, so only batch
    columns t < 64 matter.  Those depend only on x[t, c, 8*s, h, w]
    (t, s < 64) -- 3.1 MB of the 100 MB input and an 8x compute cut.
  - Critical path: 64 gru1 steps, then 64 gru2 steps, then a 6-step
    decoder GRU on batch 64.

Layout (single NeuronCore; fully transposed: gates/hidden on
partitions, batch on the free axis):
  - Input-part GEMMs batched per layer/direction as weight-stationary
    GEMMs (lhsT = W chunk, M=128 gates; rhs = data, N=512 col tiles);
    per-gate biases folded into the PSUM->SBUF eviction via
    per-partition bias columns.  Eviction rotates ACT/DVE/Pool.
  - z-gate weights/biases negated host-side so sigmoid yields (1-z)
    directly; h' = h + (1-z)*(n - h).
  - Recurrent matmul split: Whh@h(s+1) = Whh@h(s) + Whh@g(s); the
    h-part runs one step early (off the critical cycle), only the 12
    g-part matmuls (rhs = last update g) sit on the per-step cycle.
  - Per-step PSUM bank (128,512) per direction:
    cols [0:256] = r|omz logits (x-inject + h/g mms),
    cols [256:384] = hn part (+b_hh_n ones-inject);
    the decoder also uses [384:512] for its x n-part.
  - No PE transposes anywhere; h' tiles are directly the next step's
    matmul rhs; gru1's h' writes ARE the y1T storage read by gru2's
    input GEMM.  bf16 matmuls + bf16 elementwise, fp32 PSUM.
"""

import re
from contextlib import ExitStack

import ml_dtypes

import numpy as np

import concourse.bass as bass
import concourse.tile as tile
from concourse import mybir
from concourse.bass_utils import run_bass_kernel_spmd
from concourse.tile import ScopedClock
from bass_rust import VectorClock

F32 = mybir.dt.float32
BF16 = mybir.dt.bfloat16

H = 256          # GRU hidden
V = 56           # vocab / fc1 out
NB = 64          # batch (original T slots used)
NS = 64          # scan steps (original B)
G = 3 * H        # 768 gates
SN = NS * NB     # 4096

AF = mybir.ActivationFunctionType
OP = mybir.AluOpType


def _vc_ticks(vc):
    m = re.search(r"\[([0-9, ]*)\]", repr(vc))
    s = m.group(1).strip()
    return [int(x) for x in s.split(",")] if s else []


class SplitDrainTC(tile.TileContext):
    """TileContext adapted to the installed walrus, which rejects >2
    sync waits on any single instruction: excess waits are peeled onto
    same-engine NOPs at commit time, and the exit drain emits one wait
    per sync.nop."""

    MAX_WAITS = 1

    def _add_instruction(self, inst):
        si = getattr(inst, "sync_info", None)
        if si is not None and si.on_wait and len(si.on_wait) > self.MAX_WAITS:
            waits = list(si.on_wait)
            keep = waits[: self.MAX_WAITS]
            excess = waits[self.MAX_WAITS :]
            for i in range(0, len(excess), self.MAX_WAITS):
                nop = mybir.InstNoOp(
                    name=self.nc.get_next_instruction_name(),
                    engine=inst.engine,
                    bass_nofuse=True,
                    sync_info=mybir.SyncInfo(
                        on_wait=excess[i : i + self.MAX_WAITS], on_update=[]),
                )
                super()._add_instruction(nop)
            inst.sync_info = mybir.SyncInfo(on_wait=keep, on_update=si.on_update)
        super()._add_instruction(inst)

    def _drain_and_barrier(self, tick_clock, wait_clock):
        ticks = _vc_ticks(tick_clock.global_clock)
        for i, t in enumerate(ticks):
            if t > 0:
                single = VectorClock([t if j == i else 0 for j in range(len(ticks))])
                nop = self.nc.sync.nop(nofuse=True)
                wait_clock.add_sem_waits(nop.ins, ScopedClock({None: single}))
        self.nc.sync.drain()
        self.nc.all_engine_barrier()
        popped = self.nc._tile_sem_poison_stack.pop()
        assert popped is self._sem_poison
        self.nc.clear_and_free_semaphores(list(self.sems.allocated().values()))
        self.nc.all_engine_barrier()


# ---------------------------------------------------------------------------
# host-side input preparation
# ---------------------------------------------------------------------------

def prepare_inputs(inputs, nsteps=NS):
    p = {k: np.asarray(v, dtype=np.float32) for k, v in inputs.items()
         if k != "target_length"}
    x = p["x"]

    # seq'[s, t, (c,h,w)] = x[t, c, 8s, h, w];  seqT[(c,h,w), s*64+t]
    xs = x[0:NB, :, 0 : 8 * nsteps : 8, :, :]            # [t, c, s, h, w]
    seqT = np.transpose(xs, (1, 3, 4, 2, 0)).reshape(192, nsteps * NB)

    d = {"seqT0": seqT[0:128].copy(), "seqT1": seqT[128:192].copy()}

    def gru_parts(tag, wih, whh, bih, bhh, bias_row):
        wihT = wih.T.copy()
        whhT = whh.T.copy()
        # z-gate negation: sigmoid(-z_logit) = 1 - z
        wihT[:, 256:512] *= -1.0
        whhT[:, 256:512] *= -1.0
        # x-side bias: rz gets b_ih + b_hh (z part negated), n gets b_ih
        xb = np.concatenate([
            bih[0:256] + bhh[0:256],
            -(bih[256:512] + bhh[256:512]),
            bih[512:768],
        ])
        if bias_row:
            # append the bias as a K-row (rhs supplies a matching ones row)
            wihT = np.concatenate([wihT, xb[None, :]], axis=0)
        else:
            d[f"xbias{tag}"] = xb.reshape(1, 768)
        d[f"wihT{tag}"] = wihT
        d[f"whhT{tag}"] = whhT
        d[f"bhhn{tag}"] = bhh[512:768].reshape(1, 256).copy()

    gru_parts("1f", p["w_ih_1f"], p["w_hh_1f"], p["b_ih_1f"], p["b_hh_1f"], True)
    gru_parts("1b", p["w_ih_1b"], p["w_hh_1b"], p["b_ih_1b"], p["b_hh_1b"], True)
    gru_parts("2f", p["w_ih_2f"], p["w_hh_2f"], p["b_ih_2f"], p["b_hh_2f"], False)
    gru_parts("2b", p["w_ih_2b"], p["w_hh_2b"], p["b_ih_2b"], p["b_hh_2b"], False)
    # ones row for the layer-1 bias trick
    d["seqT1"] = np.concatenate(
        [d["seqT1"], np.ones((1, nsteps * NB), np.float32)], axis=0)

    # decoder
    wihdT = p["w_ih_d"].T.copy()
    whhdT = p["w_hh_d"].T.copy()
    wihdT[:, 256:512] *= -1.0
    whhdT[:, 256:512] *= -1.0
    brzd = (p["b_ih_d"][0:512] + p["b_hh_d"][0:512]).copy()
    brzd[256:512] *= -1.0
    d["wihdT"] = wihdT
    d["whhdT"] = whhdT
    d["brzd"] = brzd.reshape(1, 512)
    d["bihdn"] = p["b_ih_d"][512:768].reshape(1, 256).copy()
    d["bhhdn"] = p["b_hh_d"][512:768].reshape(1, 256).copy()

    d["wadjT"] = p["w_adj"].T.copy()                       # [512, 256]
    d["badjc"] = p["b_adj"].reshape(2, 128).T.copy()       # [128, 2]
    d["wfc1T"] = p["w_fc1"].T.copy()                       # [256, 56]
    d["bfc1c"] = p["b_fc1"].reshape(56, 1).copy()          # [56, 1]
    d["ones"] = np.ones((1, NB), np.float32)

    f32_keep = {"badjc", "bfc1c"}
    for k in list(d):
        if k not in f32_keep:
            d[k] = np.ascontiguousarray(d[k]).astype(ml_dtypes.bfloat16)
        else:
            d[k] = np.ascontiguousarray(d[k])
    return d


# ---------------------------------------------------------------------------
# device program
# ---------------------------------------------------------------------------

def build_program(nsteps=NS, tl=6):
    nc = bass.Bass("TRN2", target_bir_lowering=False, debug=False)
    sn = nsteps * NB

    dp = {}

    def din(name, shape, dtype=BF16):
        dp[name] = nc.declare_dram_parameter(name, list(shape), dtype, isOutput=False)

    din("seqT0", (128, sn))
    din("seqT1", (65, sn))
    for tag in ("1f", "1b"):
        din(f"wihT{tag}", (193, G))
    for tag in ("2f", "2b"):
        din(f"wihT{tag}", (512, G))
        din(f"xbias{tag}", (1, G))
    for tag in ("1f", "1b", "2f", "2b"):
        din(f"whhT{tag}", (H, G))
        din(f"bhhn{tag}", (1, 256))
    din("wihdT", (V, G))
    din("whhdT", (H, G))
    din("brzd", (1, 512))
    din("bihdn", (1, 256))
    din("bhhdn", (1, 256))
    din("wadjT", (512, 256))
    din("badjc", (128, 2), F32)
    din("wfc1T", (256, V))
    din("bfc1c", (V, 1), F32)
    din("ones", (1, NB))

    out_dram = nc.declare_dram_parameter("out", [tl, V, NB], F32, isOutput=True)

    with SplitDrainTC(nc) as tc:
        es = ExitStack()
        cpool = es.enter_context(tc.tile_pool(name="consts", bufs=1))

        def load(name, shape, dtype=BF16, src=None):
            t = cpool.tile(list(shape), dtype, tag=name, name=name)
            nc.sync.dma_start(out=t[:], in_=src if src is not None else dp[name][:])
            return t

        seqT = [load("seqT0", (128, sn)), load("seqT1", (65, sn))]
        WIH1 = {}
        for tag in ("1f", "1b"):
            WIH1[tag] = [
                load(f"wihT{tag}_0", (128, G), src=dp[f"wihT{tag}"][0:128, :]),
                load(f"wihT{tag}_1", (65, G), src=dp[f"wihT{tag}"][128:193, :]),
            ]
        WIH2, XB2 = {}, {}
        for tag in ("2f", "2b"):
            WIH2[tag] = [
                load(f"wihT{tag}_{k}", (128, G),
                     src=dp[f"wihT{tag}"][k * 128:(k + 1) * 128, :])
                for k in range(4)
            ]
            XB2[tag] = load(f"xbias{tag}", (1, G))
        WHH, BHHN = {}, {}
        for tag in ("1f", "1b", "2f", "2b"):
            WHH[tag] = [
                load(f"whhT{tag}_{k}", (128, G),
                     src=dp[f"whhT{tag}"][k * 128:(k + 1) * 128, :])
                for k in range(2)
            ]
            BHHN[tag] = load(f"bhhn{tag}", (1, 256))
        WIHD = load("wihdT", (V, G))
        WHHD = [load(f"whhdT_{k}", (128, G),
                     src=dp["whhdT"][k * 128:(k + 1) * 128, :]) for k in range(2)]
        BRZD = load("brzd", (1, 512))
        BIHDN = load("bihdn", (1, 256))
        BHHDN = load("bhhdn", (1, 256))
        WADJ = [load(f"wadjT_{k}", (128, 256),
                     src=dp["wadjT"][k * 128:(k + 1) * 128, :]) for k in range(4)]
        BADJC = load("badjc", (128, 2), F32)
        WFC1 = [load(f"wfc1T_{k}", (128, V),
                     src=dp["wfc1T"][k * 128:(k + 1) * 128, :]) for k in range(2)]
        BFC1C = load("bfc1c", (V, 1), F32)
        ONES = load("ones", (1, NB))

        y1T = cpool.tile([128, 4, sn], BF16, tag="y1T", name="y1T")
        zeros3 = cpool.tile([128, 2, NB], BF16, tag="zeros3", name="zeros3")
        nc.vector.memset(zeros3[:], 0.0)

        pstep = es.enter_context(tc.tile_pool(name="pstep", bufs=2, space="PSUM"))
        pgem = es.enter_context(tc.tile_pool(name="pgem", bufs=2, space="PSUM"))
        wrk = es.enter_context(tc.tile_pool(name="wrk", bufs=3))

        # ----------------------------------------------------------------
        # one BiGRU layer, transposed layout, g-split recurrence.
        # All x-parts are direct per-step matmuls into the step's PSUM
        # bank: [r0 r1 z0 z1 | hn0 hn1 | xn0 xn1] cols.
        # ----------------------------------------------------------------
        def gru_layer(tag_f, tag_b, x_mms, y_store):
            """x_mms(bank, d, s, start): emit the x-part matmuls for the
            step (opener carries start=True).
            y_store: None -> rotating h tiles (gru2); else the y1T tile
            (gru1: h' writes double as the y1 storage)."""
            tags = (tag_f, tag_b)
            col = lambda d, s: s if d == 0 else nsteps - 1 - s

            class HRef:
                def __init__(self, tile_, d, c):
                    self.t, self.d, self.c = tile_, d, c

                def full(self):
                    if self.c is None:
                        return self.t[:]
                    return self.t[:, 2 * self.d:2 * self.d + 2,
                                  self.c * NB:(self.c + 1) * NB]

                def rhs(self, ki):
                    if self.c is None:
                        return self.t[:, ki, :]
                    return self.t[:, 2 * self.d + ki,
                                  self.c * NB:(self.c + 1) * NB]

            def make_hdst(d, s):
                if y_store is None:
                    t = wrk.tile([128, 2, NB], BF16, tag=f"h2_{d}",
                                 name=f"h2_{d}")
                    return HRef(t, d, None)
                return HRef(y_store, d, col(d, s))

            hmap = {}

            def bhhn_mms(bank, d, stop):
                bt = BHHN[tags[d]]
                for ch in range(2):
                    nc.tensor.matmul(
                        bank[:, 256 + ch * NB:256 + (ch + 1) * NB],
                        bt[:, ch * 128:(ch + 1) * 128], ONES[:],
                        start=False, stop=(stop and ch == 1),
                        skip_group_check=True)

            def wh_mms(bank, d, rhs_fn, stop):
                w = WHH[tags[d]]
                for m in range(6):
                    c0 = m * NB if m < 4 else 256 + (m - 4) * NB
                    for ki in range(2):
                        nc.tensor.matmul(
                            bank[:, c0:c0 + NB],
                            w[ki][:, m * 128:(m + 1) * 128],
                            rhs_fn(ki),
                            start=False,
                            stop=(stop and m == 5 and ki == 1),
                            skip_group_check=True)

            banks = {}
            for d in (0, 1):
                b0 = pstep.tile([128, 512], F32, tag=f"A{d}", name=f"bank{d}")
                x_mms(b0, d, col(d, 0), True)
                bhhn_mms(b0, d, stop=True)
                banks[d] = b0

            for s in range(nsteps):
                # elementwise chain
                rzs, tmp, npre, nt, dmn = {}, {}, {}, {}, {}
                for d in (0, 1):
                    rzs[d] = wrk.tile([128, 4, NB], BF16, tag=f"rzs{d}",
                                      name=f"rzs{d}")
                    nc.scalar.activation(rzs[d][:], banks[d][:, 0:256],
                                         AF.Sigmoid)
                for d in (0, 1):
                    tmp[d] = wrk.tile([128, 2, NB], BF16, tag=f"tmp{d}",
                                      name=f"tmp{d}")
                    nc.vector.tensor_tensor(tmp[d][:], rzs[d][:, 0:2, :],
                                            banks[d][:, 256:384], OP.mult)
                    npre[d] = wrk.tile([128, 2, NB], BF16, tag=f"npre{d}",
                                       name=f"npre{d}")
                    nc.vector.tensor_tensor(npre[d][:], tmp[d][:],
                                            banks[d][:, 384:512], OP.add)
                for d in (0, 1):
                    nt[d] = wrk.tile([128, 2, NB], BF16, tag=f"nt{d}",
                                     name=f"nt{d}")
                    nc.scalar.activation(nt[d][:], npre[d][:], AF.Tanh)
                for d in (0, 1):
                    hc = zeros3[:] if s == 0 else hmap[d].full()
                    dmn[d] = wrk.tile([128, 2, NB], BF16, tag=f"dmn{d}",
                                      name=f"dmn{d}")
                    nc.vector.tensor_tensor(dmn[d][:], nt[d][:], hc, OP.subtract)
                for d in (0, 1):
                    g = wrk.tile([128, 2, NB], BF16, tag=f"g{d}", name=f"g{d}")
                    nc.vector.tensor_tensor(g[:], rzs[d][:, 2:4, :],
                                            dmn[d][:], OP.mult)
                    hc = zeros3[:] if s == 0 else hmap[d].full()
                    hdst = make_hdst(d, s)
                    nc.vector.tensor_tensor(hdst.full(), hc, g[:], OP.add)
                    hmap[d] = hdst
                # open + fully fill next banks (h-part rhs = just-written h)
                if s + 1 < nsteps:
                    for d in (0, 1):
                        nb = pstep.tile([128, 512], F32, tag=f"A{d}",
                                        name=f"bank{d}")
                        x_mms(nb, d, col(d, s + 1), True)
                        bhhn_mms(nb, d, stop=False)
                        wh_mms(nb, d, hmap[d].rhs, stop=True)
                        banks[d] = nb
            return hmap

        # =================== phase 1: gru1 ==============================
        def x_mms1(bank, d, c, start):
            w = WIH1["1f" if d == 0 else "1b"]
            first = True
            for m in range(6):
                c0 = m * NB if m < 4 else 384 + (m - 4) * NB
                for ki in range(2):
                    nc.tensor.matmul(
                        bank[:, c0:c0 + NB],
                        w[ki][:, m * 128:(m + 1) * 128],
                        seqT[ki][:, c * NB:(c + 1) * NB],
                        start=(start and first), stop=False,
                        skip_group_check=not (start and first))
                    first = False

        gru_layer("1f", "1b", x_mms1, y1T)

        # =================== phase 2: gru2 ==============================
        def x_mms2(bank, d, c, start):
            tag = "2f" if d == 0 else "2b"
            w = WIH2[tag]
            xb = XB2[tag]
            first = True
            for m in range(6):
                c0 = m * NB if m < 4 else 384 + (m - 4) * NB
                for ki in range(4):
                    nc.tensor.matmul(
                        bank[:, c0:c0 + NB],
                        w[ki][:, m * 128:(m + 1) * 128],
                        y1T[:, ki, c * NB:(c + 1) * NB],
                        start=(start and first), stop=False,
                        skip_group_check=not (start and first))
                    first = False
                nc.tensor.matmul(
                    bank[:, c0:c0 + NB], xb[:, m * 128:(m + 1) * 128],
                    ONES[:], start=False, stop=False, skip_group_check=True)

        h2 = gru_layer("2f", "2b", x_mms2, None)

        # =================== decoder ====================================
        # dec_h = [h2f, h2b] @ w_adj.T + b_adj   (transposed: (2x128, 64))
        pd = pgem.tile([128, 512], F32, tag="pg", name="pd")
        first = True
        for m in range(2):
            for ki in range(4):
                rhs = h2[0].rhs(ki) if ki < 2 else h2[1].rhs(ki - 2)
                nc.tensor.matmul(pd[:, m * NB:(m + 1) * NB],
                                 WADJ[ki][:, m * 128:(m + 1) * 128], rhs,
                                 start=first, stop=(m == 1 and ki == 3),
                                 skip_group_check=not first)
                first = False
        hdec = wrk.tile([128, 2, NB], BF16, tag="hdec", name="hdec")
        for m in range(2):
            nc.scalar.activation(hdec[:, m, :], pd[:, m * NB:(m + 1) * NB],
                                 AF.Identity, bias=BADJC[:, m:m + 1])
        hdec_t = hdec

        inT = None
        for t in range(tl):
            bank = pstep.tile([128, 512], F32, tag="A0", name="bankd")
            # bias injects (opener: first brzd mm)
            for m in range(4):
                nc.tensor.matmul(bank[:, m * NB:(m + 1) * NB],
                                 BRZD[:, m * 128:(m + 1) * 128], ONES[:],
                                 start=(m == 0), stop=False,
                                 skip_group_check=(m != 0))
            for ch in range(2):
                nc.tensor.matmul(bank[:, 384 + ch * NB:384 + (ch + 1) * NB],
                                 BIHDN[:, ch * 128:(ch + 1) * 128], ONES[:],
                                 start=False, stop=False, skip_group_check=True)
                nc.tensor.matmul(bank[:, 256 + ch * NB:256 + (ch + 1) * NB],
                                 BHHDN[:, ch * 128:(ch + 1) * 128], ONES[:],
                                 start=False, stop=False, skip_group_check=True)
            # x-part (input is zero at t=0)
            if inT is not None:
                for m in range(6):
                    c0 = m * NB if m < 4 else 384 + (m - 4) * NB
                    nc.tensor.matmul(bank[:, c0:c0 + NB],
                                     WIHD[:, m * 128:(m + 1) * 128], inT[:],
                                     start=False, stop=False,
                                     skip_group_check=True)
            # h-part
            for m in range(6):
                c0 = m * NB if m < 4 else 256 + (m - 4) * NB
                for ki in range(2):
                    nc.tensor.matmul(bank[:, c0:c0 + NB],
                                     WHHD[ki][:, m * 128:(m + 1) * 128],
                                     hdec_t[:, ki, :],
                                     start=False,
                                     stop=(m == 5 and ki == 1),
                                     skip_group_check=True)
            rzs = wrk.tile([128, 4, NB], BF16, tag="rzsd", name="rzsd")
            nc.scalar.activation(rzs[:], bank[:, 0:256], AF.Sigmoid)
            tmp = wrk.tile([128, 2, NB], BF16, tag="tmpd", name="tmpd")
            nc.vector.tensor_tensor(tmp[:], rzs[:, 0:2, :], bank[:, 256:384],
                                    OP.mult)
            npre = wrk.tile([128, 2, NB], BF16, tag="npred", name="npred")
            nc.vector.tensor_tensor(npre[:], tmp[:], bank[:, 384:512], OP.add)
            nt = wrk.tile([128, 2, NB], BF16, tag="ntd", name="ntd")
            nc.scalar.activation(nt[:], npre[:], AF.Tanh)
            dmn = wrk.tile([128, 2, NB], BF16, tag="dmnd", name="dmnd")
            nc.vector.tensor_tensor(dmn[:], nt[:], hdec_t[:], OP.subtract)
            g = wrk.tile([128, 2, NB], BF16, tag="gd", name="gd")
            nc.vector.tensor_tensor(g[:], rzs[:, 2:4, :], dmn[:], OP.mult)
            hnew = wrk.tile([128, 2, NB], BF16, tag="hnd", name="hnd")
            nc.gpsimd.tensor_tensor(hnew[:], hdec_t[:], g[:], OP.add)
            hdec_t = hnew

            # fc1: outT = w_fc1 @ h' + b   -> (56, 64)
            pf = pgem.tile([128, 512], F32, tag="pg", name="pf")
            for ki in range(2):
                nc.tensor.matmul(pf[0:V, 0:NB], WFC1[ki][:, 0:V],
                                 hnew[:, ki, :],
                                 start=(ki == 0), stop=(ki == 1))
            outf = wrk.tile([V, NB], F32, tag="outf", name="outf")
            nc.scalar.activation(outf[:], pf[0:V, 0:NB], AF.Identity,
                                 bias=BFC1C[:, 0:1])
            nc.sync.dma_start(out=out_dram[t], in_=outf[:])
            if t + 1 < tl:
                it = wrk.tile([V, NB], BF16, tag="inT", name="inT")
                nc.vector.tensor_copy(it[:], outf[:])
                inT = it

        es.close()

    return nc


_PROG_CACHE = {}


def _get_program(nsteps, tl):
    key = (nsteps, tl)
    if key not in _PROG_CACHE:
        _PROG_CACHE[key] = build_program(nsteps, tl)
    return _PROG_CACHE[key]


def run_device(inputs, nsteps=NS, trace=False):
    tl = int(np.asarray(inputs["target_length"]))
    nc = _get_program(nsteps, tl)
    d = prepare_inputs(inputs, nsteps)
    res = run_bass_kernel_spmd(nc, [d], [0], trace=trace)
    out = res.results[0]["out"]          # [tl, V, NB]
    full = np.ascontiguousarray(
        np.transpose(out, (2, 0, 1)).astype(np.float32))
    return full, res


def kernel(**inputs):
    return run_device(inputs)[0]


# revision 26
# speedup vs baseline: 1.2266x; 1.2266x over previous
"""Trainium2 Bass kernel for nn_DenseNet3D_89730456748628.

Reference structure (after dead-code elimination):
  - The reference builds seq (B=64, T=512, 192) and runs two BiGRUs with
    sequence axis = B (64 steps) and batch axis = T (512).  The decoder
    consumes only dec_h_all[:64] and y2 is discarded, so only batch
    columns t < 64 matter.  Those depend only on x[t, c, 8*s, h, w]
    (t, s < 64) -- 3.1 MB of the 100 MB input and an 8x compute cut.
  - Critical path: 64 gru1 steps, then 64 gru2 steps, then a 6-step
    decoder GRU on batch 64.

Layout (single NeuronCore; fully transposed: gates/hidden on
partitions, batch on the free axis):
  - Input-part GEMMs batched per layer/direction as weight-stationary
    GEMMs (lhsT = W chunk, M=128 gates; rhs = data, N=512 col tiles);
    per-gate biases folded into the PSUM->SBUF eviction via
    per-partition bias columns.  Eviction alternates ACT/DVE.
  - z-gate weights/biases negated host-side so sigmoid yields (1-z)
    directly; h' = h + (1-z)*(n - h).
  - Recurrent matmul split: Whh@h(s+1) = Whh@h(s) + Whh@g(s); the
    h-part runs one step early (off the critical cycle), only the 12
    g-part matmuls (rhs = last update g) sit on the per-step cycle.
    The dense per-step PE stream also keeps the PE p-state high.
  - Per-step PSUM bank (128,512) per direction:
    cols [0:256] = r|omz logits (x-inject + h/g mms),
    cols [256:384] = hn part (+b_hh_n ones-inject);
    the decoder also uses [384:512] for its x n-part.
  - No PE transposes anywhere; h' tiles are directly the next step's
    matmul rhs; gru1's h' writes ARE the y1T storage read by gru2's
    input GEMM.  bf16 matmuls + bf16 elementwise, fp32 PSUM.
"""

import re
from contextlib import ExitStack

import ml_dtypes

import numpy as np

import concourse.bass as bass
import concourse.tile as tile
from concourse import mybir
from concourse.bass_utils import run_bass_kernel_spmd
from concourse.tile import ScopedClock
from bass_rust import VectorClock

F32 = mybir.dt.float32
BF16 = mybir.dt.bfloat16

H = 256          # GRU hidden
V = 56           # vocab / fc1 out
NB = 64          # batch (original T slots used)
NS = 64          # scan steps (original B)
G = 3 * H        # 768 gates
SN = NS * NB     # 4096

AF = mybir.ActivationFunctionType
OP = mybir.AluOpType


def _vc_ticks(vc):
    m = re.search(r"\[([0-9, ]*)\]", repr(vc))
    s = m.group(1).strip()
    return [int(x) for x in s.split(",")] if s else []


class SplitDrainTC(tile.TileContext):
    """TileContext adapted to the installed walrus, which rejects >2
    sync waits on any single instruction: excess waits are peeled onto
    same-engine NOPs at commit time, and the exit drain emits one wait
    per sync.nop."""

    MAX_WAITS = 1

    def _add_instruction(self, inst):
        si = getattr(inst, "sync_info", None)
        if si is not None and si.on_wait and len(si.on_wait) > self.MAX_WAITS:
            waits = list(si.on_wait)
            keep = waits[: self.MAX_WAITS]
            excess = waits[self.MAX_WAITS :]
            for i in range(0, len(excess), self.MAX_WAITS):
                nop = mybir.InstNoOp(
                    name=self.nc.get_next_instruction_name(),
                    engine=inst.engine,
                    bass_nofuse=True,
                    sync_info=mybir.SyncInfo(
                        on_wait=excess[i : i + self.MAX_WAITS], on_update=[]),
                )
                super()._add_instruction(nop)
            inst.sync_info = mybir.SyncInfo(on_wait=keep, on_update=si.on_update)
        super()._add_instruction(inst)

    def _drain_and_barrier(self, tick_clock, wait_clock):
        ticks = _vc_ticks(tick_clock.global_clock)
        for i, t in enumerate(ticks):
            if t > 0:
                single = VectorClock([t if j == i else 0 for j in range(len(ticks))])
                nop = self.nc.sync.nop(nofuse=True)
                wait_clock.add_sem_waits(nop.ins, ScopedClock({None: single}))
        self.nc.sync.drain()
        self.nc.all_engine_barrier()
        popped = self.nc._tile_sem_poison_stack.pop()
        assert popped is self._sem_poison
        self.nc.clear_and_free_semaphores(list(self.sems.allocated().values()))
        self.nc.all_engine_barrier()


# ---------------------------------------------------------------------------
# host-side input preparation
# ---------------------------------------------------------------------------

def prepare_inputs(inputs, nsteps=NS):
    p = {k: np.asarray(v, dtype=np.float32) for k, v in inputs.items()
         if k != "target_length"}
    x = p["x"]

    # seq'[s, t, (c,h,w)] = x[t, c, 8s, h, w];  seqT[(c,h,w), s*64+t]
    xs = x[0:NB, :, 0 : 8 * nsteps : 8, :, :]            # [t, c, s, h, w]
    seqT = np.transpose(xs, (1, 3, 4, 2, 0)).reshape(192, nsteps * NB)

    d = {"seqT0": seqT[0:128].copy(), "seqT1": seqT[128:192].copy()}

    def gru_parts(tag, wih, whh, bih, bhh):
        wihT = wih.T.copy()
        whhT = whh.T.copy()
        # z-gate negation: sigmoid(-z_logit) = 1 - z
        wihT[:, 256:512] *= -1.0
        whhT[:, 256:512] *= -1.0
        evb = np.concatenate([
            bih[0:256] + bhh[0:256],
            -(bih[256:512] + bhh[256:512]),
            bih[512:768],
        ])
        d[f"wihT{tag}"] = wihT
        d[f"whhT{tag}"] = whhT
        d[f"evb{tag}"] = evb.reshape(6, 128).T.copy()      # [128, 6]
        d[f"bhhn{tag}"] = bhh[512:768].reshape(1, 256).copy()

    gru_parts("1f", p["w_ih_1f"], p["w_hh_1f"], p["b_ih_1f"], p["b_hh_1f"])
    gru_parts("1b", p["w_ih_1b"], p["w_hh_1b"], p["b_ih_1b"], p["b_hh_1b"])
    gru_parts("2f", p["w_ih_2f"], p["w_hh_2f"], p["b_ih_2f"], p["b_hh_2f"])
    gru_parts("2b", p["w_ih_2b"], p["w_hh_2b"], p["b_ih_2b"], p["b_hh_2b"])

    # decoder
    wihdT = p["w_ih_d"].T.copy()
    whhdT = p["w_hh_d"].T.copy()
    wihdT[:, 256:512] *= -1.0
    whhdT[:, 256:512] *= -1.0
    brzd = (p["b_ih_d"][0:512] + p["b_hh_d"][0:512]).copy()
    brzd[256:512] *= -1.0
    d["wihdT"] = wihdT
    d["whhdT"] = whhdT
    d["brzd"] = brzd.reshape(1, 512)
    d["bihdn"] = p["b_ih_d"][512:768].reshape(1, 256).copy()
    d["bhhdn"] = p["b_hh_d"][512:768].reshape(1, 256).copy()

    d["wadjT"] = p["w_adj"].T.copy()                       # [512, 256]
    d["badjc"] = p["b_adj"].reshape(2, 128).T.copy()       # [128, 2]
    d["wfc1T"] = p["w_fc1"].T.copy()                       # [256, 56]
    d["bfc1c"] = p["b_fc1"].reshape(56, 1).copy()          # [56, 1]
    d["ident"] = np.eye(128, dtype=np.float32)
    d["ones"] = np.ones((1, NB), np.float32)

    f32_keep = {"evb1f", "evb1b", "evb2f", "evb2b", "badjc", "bfc1c"}
    for k in list(d):
        if k not in f32_keep:
            d[k] = np.ascontiguousarray(d[k]).astype(ml_dtypes.bfloat16)
        else:
            d[k] = np.ascontiguousarray(d[k])
    return d


# ---------------------------------------------------------------------------
# device program
# ---------------------------------------------------------------------------

def build_program(nsteps=NS, tl=6):
    nc = bass.Bass("TRN2", target_bir_lowering=False, debug=False)
    sn = nsteps * NB

    dp = {}

    def din(name, shape, dtype=BF16):
        dp[name] = nc.declare_dram_parameter(name, list(shape), dtype, isOutput=False)

    din("seqT0", (128, sn))
    din("seqT1", (64, sn))
    for tag in ("1f", "1b"):
        din(f"wihT{tag}", (192, G))
    for tag in ("2f", "2b"):
        din(f"wihT{tag}", (512, G))
    for tag in ("1f", "1b", "2f", "2b"):
        din(f"whhT{tag}", (H, G))
        din(f"evb{tag}", (128, 6), F32)
        din(f"bhhn{tag}", (1, 256))
    din("wihdT", (V, G))
    din("whhdT", (H, G))
    din("brzd", (1, 512))
    din("bihdn", (1, 256))
    din("bhhdn", (1, 256))
    din("wadjT", (512, 256))
    din("badjc", (128, 2), F32)
    din("wfc1T", (256, V))
    din("bfc1c", (V, 1), F32)
    din("ident", (128, 128))
    din("ones", (1, NB))

    out_dram = nc.declare_dram_parameter("out", [tl, V, NB], F32, isOutput=True)

    with SplitDrainTC(nc) as tc:
        es = ExitStack()
        cpool = es.enter_context(tc.tile_pool(name="consts", bufs=1))

        def load(name, shape, dtype=BF16, src=None):
            t = cpool.tile(list(shape), dtype, tag=name, name=name)
            nc.sync.dma_start(out=t[:], in_=src if src is not None else dp[name][:])
            return t

        seqT = [load("seqT0", (128, sn)), load("seqT1", (64, sn))]
        WIH1 = {}
        for tag in ("1f", "1b"):
            WIH1[tag] = [
                load(f"wihT{tag}_0", (128, G), src=dp[f"wihT{tag}"][0:128, :]),
                load(f"wihT{tag}_1", (64, G), src=dp[f"wihT{tag}"][128:192, :]),
            ]
        WIH2 = {}
        for tag in ("2f", "2b"):
            WIH2[tag] = [
                load(f"wihT{tag}_{k}", (128, G),
                     src=dp[f"wihT{tag}"][k * 128:(k + 1) * 128, :])
                for k in range(4)
            ]
        WHH, EVB, BHHN = {}, {}, {}
        for tag in ("1f", "1b", "2f", "2b"):
            WHH[tag] = [
                load(f"whhT{tag}_{k}", (128, G),
                     src=dp[f"whhT{tag}"][k * 128:(k + 1) * 128, :])
                for k in range(2)
            ]
            EVB[tag] = load(f"evb{tag}", (128, 6), F32)
            BHHN[tag] = load(f"bhhn{tag}", (1, 256))
        WIHD = load("wihdT", (V, G))
        WHHD = [load(f"whhdT_{k}", (128, G),
                     src=dp["whhdT"][k * 128:(k + 1) * 128, :]) for k in range(2)]
        BRZD = load("brzd", (1, 512))
        BIHDN = load("bihdn", (1, 256))
        BHHDN = load("bhhdn", (1, 256))
        WADJ = [load(f"wadjT_{k}", (128, 256),
                     src=dp["wadjT"][k * 128:(k + 1) * 128, :]) for k in range(4)]
        BADJC = load("badjc", (128, 2), F32)
        WFC1 = [load(f"wfc1T_{k}", (128, V),
                     src=dp["wfc1T"][k * 128:(k + 1) * 128, :]) for k in range(2)]
        BFC1C = load("bfc1c", (V, 1), F32)
        IDENT = load("ident", (128, 128))
        ONES = load("ones", (1, NB))

        # big SBUF storage.  xTf/xTb are reused by both layers (the gru2
        # input GEMM overwrites them only after gru1's recurrence has
        # consumed them -- enforced by tile dependency tracking).
        xTf = cpool.tile([128, 6, sn], BF16, tag="xTf", name="xTf")
        xTb = cpool.tile([128, 6, sn], BF16, tag="xTb", name="xTb")
        y1T = cpool.tile([128, 4, sn], BF16, tag="y1T", name="y1T")
        zeros3 = cpool.tile([128, 2, NB], BF16, tag="zeros3", name="zeros3")
        nc.vector.memset(zeros3[:], 0.0)

        pstep = es.enter_context(tc.tile_pool(name="pstep", bufs=2, space="PSUM"))
        pgem = es.enter_context(tc.tile_pool(name="pgem", bufs=2, space="PSUM"))
        wrk = es.enter_context(tc.tile_pool(name="wrk", bufs=3))

        # ----------------------------------------------------------------
        # batched input-part GEMM:
        #   xdst[:, m, cb*512:...] = (sum_ki lhs[ki].T @ rhs(ki, cb))[Mtile m] + evb[:, m]
        # cb_order lets the backward direction land its first-consumed
        # (high) column blocks early so the recurrence can overlap.
        # ----------------------------------------------------------------
        def xgemm(xdst, lhs_chunks, rhs_fn, evb, ecnt0=0, cb_order=None):
            nblk = sn // 512
            ecnt = ecnt0
            if cb_order is None:
                cb_order = list(range(nblk))
            for cb in cb_order:
                for m in range(6):
                    pg = pgem.tile([128, 512], F32, tag="pg", name="pg")
                    nk = len(lhs_chunks)
                    for ki in range(nk):
                        nc.tensor.matmul(
                            pg[:], lhs_chunks[ki][:, m * 128:(m + 1) * 128],
                            rhs_fn(ki, cb),
                            start=(ki == 0), stop=(ki == nk - 1))
                    dst = xdst[:, m, cb * 512:(cb + 1) * 512]
                    e = ecnt % 2
                    ecnt += 1
                    if e == 0:
                        nc.scalar.activation(dst, pg[:], AF.Identity,
                                             bias=evb[:, m:m + 1])
                    else:
                        nc.vector.tensor_scalar(dst, pg[:], evb[:, m:m + 1],
                                                None, OP.add)
            return ecnt

        # ----------------------------------------------------------------
        # one BiGRU layer, transposed layout, g-split recurrence
        # ----------------------------------------------------------------
        def gru_layer(tag_f, tag_b, xT, y_store):
            """xT[d] = input-part tensor (128, 6, sn) for dir d.
            y_store: None -> rotating h tiles (gru2); else the y1T tile
            (gru1: h' writes double as the y1 storage)."""
            tags = (tag_f, tag_b)
            col = lambda d, s: s if d == 0 else nsteps - 1 - s

            class HRef:
                def __init__(self, tile_, d, c):
                    self.t, self.d, self.c = tile_, d, c

                def full(self):
                    if self.c is None:
                        return self.t[:]
                    return self.t[:, 2 * self.d:2 * self.d + 2,
                                  self.c * NB:(self.c + 1) * NB]

                def rhs(self, ki):
                    if self.c is None:
                        return self.t[:, ki, :]
                    return self.t[:, 2 * self.d + ki,
                                  self.c * NB:(self.c + 1) * NB]

            def make_hdst(d, s):
                if y_store is None:
                    t = wrk.tile([128, 2, NB], BF16, tag=f"h2_{d}",
                                 name=f"h2_{d}")
                    return HRef(t, d, None)
                return HRef(y_store, d, col(d, s))

            hmap = {}

            def inject(bank, d, s, start):
                c = col(d, s)
                nc.tensor.matmul(bank[:, 0:256], IDENT[:],
                                 xT[d][:, 0:4, c * NB:(c + 1) * NB],
                                 start=start, stop=False,
                                 skip_group_check=not start)

            def bhhn_mms(bank, d, stop):
                bt = BHHN[tags[d]]
                for ch in range(2):
                    nc.tensor.matmul(
                        bank[:, 256 + ch * NB:256 + (ch + 1) * NB],
                        bt[:, ch * 128:(ch + 1) * 128], ONES[:],
                        start=False, stop=(stop and ch == 1),
                        skip_group_check=True)

            def wh_mms(bank, d, rhs_fn, stop):
                w = WHH[tags[d]]
                for m in range(6):
                    c0 = m * NB if m < 4 else 256 + (m - 4) * NB
                    for ki in range(2):
                        nc.tensor.matmul(
                            bank[:, c0:c0 + NB],
                            w[ki][:, m * 128:(m + 1) * 128],
                            rhs_fn(ki),
                            start=False,
                            stop=(stop and m == 5 and ki == 1),
                            skip_group_check=True)

            banks = {}
            for d in (0, 1):
                b0 = pstep.tile([128, 512], F32, tag=f"A{d}", name=f"bank{d}")
                inject(b0, d, 0, start=True)
                bhhn_mms(b0, d, stop=True)
                banks[d] = b0
            gt = {0: None, 1: None}

            for s in range(nsteps):
                # close current banks with g-part matmuls
                if s >= 1:
                    for d in (0, 1):
                        gtile = gt[d]
                        wh_mms(banks[d], d, lambda ki, _t=gtile: _t[:, ki, :],
                               stop=True)
                # open next banks; h-part runs one step ahead
                nbanks = {}
                if s + 1 < nsteps:
                    for d in (0, 1):
                        nb = pstep.tile([128, 512], F32, tag=f"A{d}",
                                        name=f"bank{d}")
                        inject(nb, d, s + 1, start=True)
                        bhhn_mms(nb, d, stop=False)
                        if s >= 1:
                            wh_mms(nb, d, hmap[d].rhs, stop=False)
                        nbanks[d] = nb
                # elementwise chain
                rzs, tmp, npre, nt, dmn = {}, {}, {}, {}, {}
                for d in (0, 1):
                    rzs[d] = wrk.tile([128, 4, NB], BF16, tag=f"rzs{d}",
                                      name=f"rzs{d}")
                    nc.scalar.activation(rzs[d][:], banks[d][:, 0:256],
                                         AF.Sigmoid)
                for d in (0, 1):
                    tmp[d] = wrk.tile([128, 2, NB], BF16, tag=f"tmp{d}",
                                      name=f"tmp{d}")
                    nc.vector.tensor_tensor(tmp[d][:], rzs[d][:, 0:2, :],
                                            banks[d][:, 256:384], OP.mult)
                    c = col(d, s)
                    npre[d] = wrk.tile([128, 2, NB], BF16, tag=f"npre{d}",
                                       name=f"npre{d}")
                    nc.vector.tensor_tensor(npre[d][:], tmp[d][:],
                                            xT[d][:, 4:6, c * NB:(c + 1) * NB],
                                            OP.add)
                for d in (0, 1):
                    nt[d] = wrk.tile([128, 2, NB], BF16, tag=f"nt{d}",
                                     name=f"nt{d}")
                    nc.scalar.activation(nt[d][:], npre[d][:], AF.Tanh)
                for d in (0, 1):
                    hc = zeros3[:] if s == 0 else hmap[d].full()
                    dmn[d] = wrk.tile([128, 2, NB], BF16, tag=f"dmn{d}",
                                      name=f"dmn{d}")
                    nc.vector.tensor_tensor(dmn[d][:], nt[d][:], hc, OP.subtract)
                for d in (0, 1):
                    g = wrk.tile([128, 2, NB], BF16, tag=f"g{d}", name=f"g{d}")
                    nc.vector.tensor_tensor(g[:], rzs[d][:, 2:4, :],
                                            dmn[d][:], OP.mult)
                    gt[d] = g
                for d in (0, 1):
                    hc = zeros3[:] if s == 0 else hmap[d].full()
                    hdst = make_hdst(d, s)
                    nc.gpsimd.tensor_tensor(hdst.full(), hc, gt[d][:], OP.add)
                    hmap[d] = hdst
                banks = nbanks
            return hmap

        # =================== phase 1: gru1 ==============================
        def seq_rhs(ki, cb):
            return seqT[ki][:, cb * 512:(cb + 1) * 512]

        rev = list(range(sn // 512 - 1, -1, -1))
        ec = xgemm(xTf, WIH1["1f"], seq_rhs, EVB["1f"])
        ec = xgemm(xTb, WIH1["1b"], seq_rhs, EVB["1b"], ec, cb_order=rev)

        h1 = gru_layer("1f", "1b", (xTf, xTb), y1T)

        # =================== phase 2: gru2 ==============================
        def y1_rhs(ki, cb):
            return y1T[:, ki, cb * 512:(cb + 1) * 512]

        ec = xgemm(xTf, WIH2["2f"], y1_rhs, EVB["2f"], ec)
        ec = xgemm(xTb, WIH2["2b"], y1_rhs, EVB["2b"], ec, cb_order=rev)

        h2 = gru_layer("2f", "2b", (xTf, xTb), None)

        # =================== decoder ====================================
        # dec_h = [h2f, h2b] @ w_adj.T + b_adj   (transposed: (2x128, 64))
        pd = pgem.tile([128, 512], F32, tag="pg", name="pd")
        first = True
        for m in range(2):
            for ki in range(4):
                rhs = h2[0].rhs(ki) if ki < 2 else h2[1].rhs(ki - 2)
                nc.tensor.matmul(pd[:, m * NB:(m + 1) * NB],
                                 WADJ[ki][:, m * 128:(m + 1) * 128], rhs,
                                 start=first, stop=(m == 1 and ki == 3),
                                 skip_group_check=not first)
                first = False
        hdec = wrk.tile([128, 2, NB], BF16, tag="hdec", name="hdec")
        for m in range(2):
            nc.scalar.activation(hdec[:, m, :], pd[:, m * NB:(m + 1) * NB],
                                 AF.Identity, bias=BADJC[:, m:m + 1])
        hdec_t = hdec

        inT = None
        for t in range(tl):
            bank = pstep.tile([128, 512], F32, tag="A0", name="bankd")
            # bias injects (opener: first brzd mm)
            for m in range(4):
                nc.tensor.matmul(bank[:, m * NB:(m + 1) * NB],
                                 BRZD[:, m * 128:(m + 1) * 128], ONES[:],
                                 start=(m == 0), stop=False,
                                 skip_group_check=(m != 0))
            for ch in range(2):
                nc.tensor.matmul(bank[:, 384 + ch * NB:384 + (ch + 1) * NB],
                                 BIHDN[:, ch * 128:(ch + 1) * 128], ONES[:],
                                 start=False, stop=False, skip_group_check=True)
                nc.tensor.matmul(bank[:, 256 + ch * NB:256 + (ch + 1) * NB],
                                 BHHDN[:, ch * 128:(ch + 1) * 128], ONES[:],
                                 start=False, stop=False, skip_group_check=True)
            # x-part (input is zero at t=0)
            if inT is not None:
                for m in range(6):
                    c0 = m * NB if m < 4 else 384 + (m - 4) * NB
                    nc.tensor.matmul(bank[:, c0:c0 + NB],
                                     WIHD[:, m * 128:(m + 1) * 128], inT[:],
                                     start=False, stop=False,
                                     skip_group_check=True)
            # h-part
            for m in range(6):
                c0 = m * NB if m < 4 else 256 + (m - 4) * NB
                for ki in range(2):
                    nc.tensor.matmul(bank[:, c0:c0 + NB],
                                     WHHD[ki][:, m * 128:(m + 1) * 128],
                                     hdec_t[:, ki, :],
                                     start=False,
                                     stop=(m == 5 and ki == 1),
                                     skip_group_check=True)
            rzs = wrk.tile([128, 4, NB], BF16, tag="rzsd", name="rzsd")
            nc.scalar.activation(rzs[:], bank[:, 0:256], AF.Sigmoid)
            tmp = wrk.tile([128, 2, NB], BF16, tag="tmpd", name="tmpd")
            nc.vector.tensor_tensor(tmp[:], rzs[:, 0:2, :], bank[:, 256:384],
                                    OP.mult)
            npre = wrk.tile([128, 2, NB], BF16, tag="npred", name="npred")
            nc.vector.tensor_tensor(npre[:], tmp[:], bank[:, 384:512], OP.add)
            nt = wrk.tile([128, 2, NB], BF16, tag="ntd", name="ntd")
            nc.scalar.activation(nt[:], npre[:], AF.Tanh)
            dmn = wrk.tile([128, 2, NB], BF16, tag="dmnd", name="dmnd")
            nc.vector.tensor_tensor(dmn[:], nt[:], hdec_t[:], OP.subtract)
            g = wrk.tile([128, 2, NB], BF16, tag="gd", name="gd")
            nc.vector.tensor_tensor(g[:], rzs[:, 2:4, :], dmn[:], OP.mult)
            hnew = wrk.tile([128, 2, NB], BF16, tag="hnd", name="hnd")
            nc.gpsimd.tensor_tensor(hnew[:], hdec_t[:], g[:], OP.add)
            hdec_t = hnew

            # fc1: outT = w_fc1 @ h' + b   -> (56, 64)
            pf = pgem.tile([128, 512], F32, tag="pg", name="pf")
            for ki in range(2):
                nc.tensor.matmul(pf[0:V, 0:NB], WFC1[ki][:, 0:V],
                                 hnew[:, ki, :],
                                 start=(ki == 0), stop=(ki == 1))
            outf = wrk.tile([V, NB], F32, tag="outf", name="outf")
            nc.scalar.activation(outf[:], pf[0:V, 0:NB], AF.Identity,
                                 bias=BFC1C[:, 0:1])
            nc.sync.dma_start(out=out_dram[t], in_=outf[:])
            if t + 1 < tl:
                it = wrk.tile([V, NB], BF16, tag="inT", name="inT")
                nc.vector.tensor_copy(it[:], outf[:])
                inT = it

        es.close()

    return nc


_PROG_CACHE = {}


def _get_program(nsteps, tl):
    key = (nsteps, tl)
    if key not in _PROG_CACHE:
        _PROG_CACHE[key] = build_program(nsteps, tl)
    return _PROG_CACHE[key]


def run_device(inputs, nsteps=NS, trace=False):
    tl = int(np.asarray(inputs["target_length"]))
    nc = _get_program(nsteps, tl)
    d = prepare_inputs(inputs, nsteps)
    res = run_bass_kernel_spmd(nc, [d], [0], trace=trace)
    out = res.results[0]["out"]          # [tl, V, NB]
    full = np.ascontiguousarray(
        np.transpose(out, (2, 0, 1)).astype(np.float32))
    return full, res


def kernel(**inputs):
    return run_device(inputs)[0]


# revision 33
# speedup vs baseline: 1.4185x; 1.1565x over previous
"""Trainium2 Bass kernel for nn_DenseNet3D_89730456748628.

Reference structure (after dead-code elimination):
  - The reference builds seq (B=64, T=512, 192) and runs two BiGRUs with
    sequence axis = B (64 steps) and batch axis = T (512).  The decoder
    consumes only dec_h_all[:64] and y2 is discarded, so only batch
    columns t < 64 matter.  Those depend only on x[t, c, 8*s, h, w]
    (t, s < 64) -- 3.1 MB of the 100 MB input and an 8x compute cut.
  - Critical path: 64 gru1 steps, then 64 gru2 steps, then a 6-step
    decoder GRU on batch 64.

Layout (single NeuronCore; fully transposed: gates/hidden on
partitions, batch on the free axis):
  - Input-part GEMMs batched per layer/direction as weight-stationary
    GEMMs (lhsT = W chunk, M=128 gates; rhs = data, N=512 col tiles);
    per-gate biases folded into the PSUM->SBUF eviction via
    per-partition bias columns.  Eviction alternates ACT/DVE.
  - z-gate weights/biases negated host-side so sigmoid yields (1-z)
    directly; h' = h + (1-z)*(n - h).
  - Recurrent matmul split: Whh@h(s+1) = Whh@h(s) + Whh@g(s); the
    h-part runs one step early (off the critical cycle), only the 12
    g-part matmuls (rhs = last update g) sit on the per-step cycle.
    The dense per-step PE stream also keeps the PE p-state high.
  - Per-step PSUM bank (128,512) per direction:
    cols [0:256] = r|omz logits (x-inject + h/g mms),
    cols [256:384] = hn part (+b_hh_n ones-inject);
    the decoder also uses [384:512] for its x n-part.
  - No PE transposes anywhere; h' tiles are directly the next step's
    matmul rhs; gru1's h' writes ARE the y1T storage read by gru2's
    input GEMM.  bf16 matmuls + bf16 elementwise, fp32 PSUM.
"""

import re
from contextlib import ExitStack

import ml_dtypes

import numpy as np

import concourse.bass as bass
import concourse.tile as tile
from concourse import mybir
from concourse.bass_utils import run_bass_kernel_spmd
from concourse.tile import ScopedClock
from bass_rust import VectorClock

F32 = mybir.dt.float32
BF16 = mybir.dt.bfloat16

H = 256          # GRU hidden
V = 56           # vocab / fc1 out
NB = 64          # batch (original T slots used)
NS = 64          # scan steps (original B)
G = 3 * H        # 768 gates
SN = NS * NB     # 4096

AF = mybir.ActivationFunctionType
OP = mybir.AluOpType


def _vc_ticks(vc):
    m = re.search(r"\[([0-9, ]*)\]", repr(vc))
    s = m.group(1).strip()
    return [int(x) for x in s.split(",")] if s else []


class SplitDrainTC(tile.TileContext):
    """TileContext adapted to the installed walrus, which rejects >2
    sync waits on any single instruction: excess waits are peeled onto
    same-engine NOPs at commit time, and the exit drain emits one wait
    per sync.nop."""

    MAX_WAITS = 1

    def _add_instruction(self, inst):
        si = getattr(inst, "sync_info", None)
        if si is not None and si.on_wait and len(si.on_wait) > self.MAX_WAITS:
            waits = list(si.on_wait)
            keep = waits[: self.MAX_WAITS]
            excess = waits[self.MAX_WAITS :]
            for i in range(0, len(excess), self.MAX_WAITS):
                nop = mybir.InstNoOp(
                    name=self.nc.get_next_instruction_name(),
                    engine=inst.engine,
                    bass_nofuse=True,
                    sync_info=mybir.SyncInfo(
                        on_wait=excess[i : i + self.MAX_WAITS], on_update=[]),
                )
                super()._add_instruction(nop)
            inst.sync_info = mybir.SyncInfo(on_wait=keep, on_update=si.on_update)
        super()._add_instruction(inst)

    def _drain_and_barrier(self, tick_clock, wait_clock):
        ticks = _vc_ticks(tick_clock.global_clock)
        for i, t in enumerate(ticks):
            if t > 0:
                single = VectorClock([t if j == i else 0 for j in range(len(ticks))])
                nop = self.nc.sync.nop(nofuse=True)
                wait_clock.add_sem_waits(nop.ins, ScopedClock({None: single}))
        self.nc.sync.drain()
        self.nc.all_engine_barrier()
        popped = self.nc._tile_sem_poison_stack.pop()
        assert popped is self._sem_poison
        self.nc.clear_and_free_semaphores(list(self.sems.allocated().values()))
        self.nc.all_engine_barrier()


# ---------------------------------------------------------------------------
# host-side input preparation
# ---------------------------------------------------------------------------

def prepare_inputs(inputs, nsteps=NS):
    p = {k: np.asarray(v, dtype=np.float32) for k, v in inputs.items()
         if k != "target_length"}
    x = p["x"]

    # seq'[s, t, (c,h,w)] = x[t, c, 8s, h, w];  seqT[(c,h,w), s*64+t]
    xs = x[0:NB, :, 0 : 8 * nsteps : 8, :, :]            # [t, c, s, h, w]
    seqT = np.transpose(xs, (1, 3, 4, 2, 0)).reshape(192, nsteps * NB)

    d = {"seqT0": seqT[0:128].copy(), "seqT1": seqT[128:192].copy()}

    def gru_parts(tag, wih, whh, bih, bhh, bias_row):
        wihT = wih.T.copy()
        whhT = whh.T.copy()
        # z-gate negation: sigmoid(-z_logit) = 1 - z
        wihT[:, 256:512] *= -1.0
        whhT[:, 256:512] *= -1.0
        evb = np.concatenate([
            bih[0:256] + bhh[0:256],
            -(bih[256:512] + bhh[256:512]),
            bih[512:768],
        ])
        if bias_row:
            # bias rides as an extra K row (rhs supplies a ones row)
            wihT = np.concatenate([wihT, evb[None, :]], axis=0)
        else:
            d[f"evb{tag}"] = evb.reshape(6, 128).T.copy()  # [128, 6]
        d[f"wihT{tag}"] = wihT
        d[f"whhT{tag}"] = whhT
        d[f"bhhn{tag}"] = bhh[512:768].reshape(1, 256).copy()

    gru_parts("1f", p["w_ih_1f"], p["w_hh_1f"], p["b_ih_1f"], p["b_hh_1f"], True)
    gru_parts("1b", p["w_ih_1b"], p["w_hh_1b"], p["b_ih_1b"], p["b_hh_1b"], True)
    gru_parts("2f", p["w_ih_2f"], p["w_hh_2f"], p["b_ih_2f"], p["b_hh_2f"], False)
    gru_parts("2b", p["w_ih_2b"], p["w_hh_2b"], p["b_ih_2b"], p["b_hh_2b"], False)
    d["seqT1"] = np.concatenate(
        [d["seqT1"], np.ones((1, nsteps * NB), np.float32)], axis=0)

    # decoder
    wihdT = p["w_ih_d"].T.copy()
    whhdT = p["w_hh_d"].T.copy()
    wihdT[:, 256:512] *= -1.0
    whhdT[:, 256:512] *= -1.0
    brzd = (p["b_ih_d"][0:512] + p["b_hh_d"][0:512]).copy()
    brzd[256:512] *= -1.0
    d["wihdT"] = wihdT
    d["whhdT"] = whhdT
    d["brzd"] = brzd.reshape(1, 512)
    d["bihdn"] = p["b_ih_d"][512:768].reshape(1, 256).copy()
    d["bhhdn"] = p["b_hh_d"][512:768].reshape(1, 256).copy()

    d["wadjT"] = p["w_adj"].T.copy()                       # [512, 256]
    d["badjc"] = p["b_adj"].reshape(2, 128).T.copy()       # [128, 2]
    d["wfc1T"] = p["w_fc1"].T.copy()                       # [256, 56]
    d["bfc1c"] = p["b_fc1"].reshape(56, 1).copy()          # [56, 1]
    d["ident"] = np.eye(128, dtype=np.float32)
    d["ones"] = np.ones((1, NB), np.float32)

    f32_keep = {"evb1f", "evb1b", "evb2f", "evb2b", "badjc", "bfc1c"}
    for k in list(d):
        if k not in f32_keep:
            d[k] = np.ascontiguousarray(d[k]).astype(ml_dtypes.bfloat16)
        else:
            d[k] = np.ascontiguousarray(d[k])
    return d


# ---------------------------------------------------------------------------
# device program
# ---------------------------------------------------------------------------

def build_program(nsteps=NS, tl=6):
    nc = bass.Bass("TRN2", target_bir_lowering=False, debug=False)
    sn = nsteps * NB

    dp = {}

    def din(name, shape, dtype=BF16):
        dp[name] = nc.declare_dram_parameter(name, list(shape), dtype, isOutput=False)

    din("seqT0", (128, sn))
    din("seqT1", (65, sn))
    for tag in ("1f", "1b"):
        din(f"wihT{tag}", (193, G))
    for tag in ("2f", "2b"):
        din(f"wihT{tag}", (512, G))
        din(f"evb{tag}", (128, 6), F32)
    for tag in ("1f", "1b", "2f", "2b"):
        din(f"whhT{tag}", (H, G))
        din(f"bhhn{tag}", (1, 256))
    din("wihdT", (V, G))
    din("whhdT", (H, G))
    din("brzd", (1, 512))
    din("bihdn", (1, 256))
    din("bhhdn", (1, 256))
    din("wadjT", (512, 256))
    din("badjc", (128, 2), F32)
    din("wfc1T", (256, V))
    din("bfc1c", (V, 1), F32)
    din("ident", (128, 128))
    din("ones", (1, NB))

    out_dram = nc.declare_dram_parameter("out", [tl, V, NB], F32, isOutput=True)

    with SplitDrainTC(nc) as tc:
        es = ExitStack()
        cpool = es.enter_context(tc.tile_pool(name="consts", bufs=1))

        def load(name, shape, dtype=BF16, src=None):
            t = cpool.tile(list(shape), dtype, tag=name, name=name)
            nc.sync.dma_start(out=t[:], in_=src if src is not None else dp[name][:])
            return t

        seqT = [load("seqT0", (128, sn)), load("seqT1", (65, sn))]
        WIH1 = {}
        for tag in ("1f", "1b"):
            WIH1[tag] = [
                load(f"wihT{tag}_0", (128, G), src=dp[f"wihT{tag}"][0:128, :]),
                load(f"wihT{tag}_1", (65, G), src=dp[f"wihT{tag}"][128:193, :]),
            ]
        WIH2, EVB = {}, {}
        for tag in ("2f", "2b"):
            WIH2[tag] = [
                load(f"wihT{tag}_{k}", (128, G),
                     src=dp[f"wihT{tag}"][k * 128:(k + 1) * 128, :])
                for k in range(4)
            ]
            EVB[tag] = load(f"evb{tag}", (128, 6), F32)
        WHH, BHHN = {}, {}
        for tag in ("1f", "1b", "2f", "2b"):
            WHH[tag] = [
                load(f"whhT{tag}_{k}", (128, G),
                     src=dp[f"whhT{tag}"][k * 128:(k + 1) * 128, :])
                for k in range(2)
            ]
            BHHN[tag] = load(f"bhhn{tag}", (1, 256))
        WIHD = load("wihdT", (V, G))
        WHHD = [load(f"whhdT_{k}", (128, G),
                     src=dp["whhdT"][k * 128:(k + 1) * 128, :]) for k in range(2)]
        BRZD = load("brzd", (1, 512))
        BIHDN = load("bihdn", (1, 256))
        BHHDN = load("bhhdn", (1, 256))
        WADJ = [load(f"wadjT_{k}", (128, 256),
                     src=dp["wadjT"][k * 128:(k + 1) * 128, :]) for k in range(4)]
        BADJC = load("badjc", (128, 2), F32)
        WFC1 = [load(f"wfc1T_{k}", (128, V),
                     src=dp["wfc1T"][k * 128:(k + 1) * 128, :]) for k in range(2)]
        BFC1C = load("bfc1c", (V, 1), F32)
        IDENT = load("ident", (128, 128))
        ONES = load("ones", (1, NB))

        # big SBUF storage.  xTf/xTb are reused by both layers (the gru2
        # input GEMM overwrites them only after gru1's recurrence has
        # consumed them -- enforced by tile dependency tracking).
        xTf = cpool.tile([128, 6, sn], BF16, tag="xTf", name="xTf")
        xTb = cpool.tile([128, 6, sn], BF16, tag="xTb", name="xTb")
        y1T = cpool.tile([128, 4, sn], BF16, tag="y1T", name="y1T")
        zeros3 = cpool.tile([128, 2, NB], BF16, tag="zeros3", name="zeros3")
        nc.vector.memset(zeros3[:], 0.0)

        pstep = es.enter_context(tc.tile_pool(name="pstep", bufs=2, space="PSUM"))
        pgem = es.enter_context(tc.tile_pool(name="pgem", bufs=2, space="PSUM"))
        wrk = es.enter_context(tc.tile_pool(name="wrk", bufs=3))

        # ----------------------------------------------------------------
        # batched input-part GEMM:
        #   xdst[:, m, cb*512:...] = (sum_ki lhs[ki].T @ rhs(ki, cb))[Mtile m] + evb[:, m]
        # cb_order lets the backward direction land its first-consumed
        # (high) column blocks early so the recurrence can overlap.
        # ----------------------------------------------------------------
        def xgemm(xdst, lhs_chunks, rhs_fn, evb, ecnt0=0, cb_order=None):
            nblk = sn // 512
            ecnt = ecnt0
            if cb_order is None:
                cb_order = list(range(nblk))
            for cb in cb_order:
                for m in range(6):
                    pg = pgem.tile([128, 512], F32, tag="pg", name="pg")
                    nk = len(lhs_chunks)
                    for ki in range(nk):
                        nc.tensor.matmul(
                            pg[:], lhs_chunks[ki][:, m * 128:(m + 1) * 128],
                            rhs_fn(ki, cb),
                            start=(ki == 0), stop=(ki == nk - 1))
                    dst = xdst[:, m, cb * 512:(cb + 1) * 512]
                    e = ecnt % 2
                    ecnt += 1
                    if e == 0:
                        nc.scalar.activation(dst, pg[:], AF.Identity,
                                             bias=evb[:, m:m + 1])
                    else:
                        nc.vector.tensor_scalar(dst, pg[:], evb[:, m:m + 1],
                                                None, OP.add)
            return ecnt

        # ----------------------------------------------------------------
        # one BiGRU layer, transposed layout, g-split recurrence
        # ----------------------------------------------------------------
        def gru_layer(tag_f, tag_b, y_store, xT=None, x_mms=None):
            """x-part: either xT[d] tensors (inject + SBUF n-read) or an
            x_mms(bank, d, c, start) callback emitting direct matmuls
            (rz -> cols [0:256], xn -> [384:512]).
            y_store: None -> rotating h tiles (gru2); else the y1T tile
            (gru1: h' writes double as the y1 storage)."""
            tags = (tag_f, tag_b)
            col = lambda d, s: s if d == 0 else nsteps - 1 - s

            class HRef:
                def __init__(self, tile_, d, c):
                    self.t, self.d, self.c = tile_, d, c

                def full(self):
                    if self.c is None:
                        return self.t[:]
                    return self.t[:, 2 * self.d:2 * self.d + 2,
                                  self.c * NB:(self.c + 1) * NB]

                def rhs(self, ki):
                    if self.c is None:
                        return self.t[:, ki, :]
                    return self.t[:, 2 * self.d + ki,
                                  self.c * NB:(self.c + 1) * NB]

            def make_hdst(d, s):
                if y_store is None:
                    t = wrk.tile([128, 2, NB], BF16, tag=f"h2_{d}",
                                 name=f"h2_{d}")
                    return HRef(t, d, None)
                return HRef(y_store, d, col(d, s))

            hmap = {}

            def inject(bank, d, s, start):
                c = col(d, s)
                if x_mms is not None:
                    x_mms(bank, d, c, start)
                    return
                nc.tensor.matmul(bank[:, 0:256], IDENT[:],
                                 xT[d][:, 0:4, c * NB:(c + 1) * NB],
                                 start=start, stop=False,
                                 skip_group_check=not start)

            def bhhn_mms(bank, d, stop):
                bt = BHHN[tags[d]]
                for ch in range(2):
                    nc.tensor.matmul(
                        bank[:, 256 + ch * NB:256 + (ch + 1) * NB],
                        bt[:, ch * 128:(ch + 1) * 128], ONES[:],
                        start=False, stop=(stop and ch == 1),
                        skip_group_check=True)

            def wh_mms(bank, d, rhs_fn, stop):
                w = WHH[tags[d]]
                for m in range(6):
                    c0 = m * NB if m < 4 else 256 + (m - 4) * NB
                    for ki in range(2):
                        nc.tensor.matmul(
                            bank[:, c0:c0 + NB],
                            w[ki][:, m * 128:(m + 1) * 128],
                            rhs_fn(ki),
                            start=False,
                            stop=(stop and m == 5 and ki == 1),
                            skip_group_check=True)

            banks = {}
            for d in (0, 1):
                b0 = pstep.tile([128, 512], F32, tag=f"A{d}", name=f"bank{d}")
                inject(b0, d, 0, start=True)
                bhhn_mms(b0, d, stop=True)
                banks[d] = b0
            gt = {0: None, 1: None}

            for s in range(nsteps):
                # close current banks with g-part matmuls
                if s >= 1:
                    for d in (0, 1):
                        gtile = gt[d]
                        wh_mms(banks[d], d, lambda ki, _t=gtile: _t[:, ki, :],
                               stop=True)
                # open next banks; h-part runs one step ahead
                nbanks = {}
                if s + 1 < nsteps:
                    for d in (0, 1):
                        nb = pstep.tile([128, 512], F32, tag=f"A{d}",
                                        name=f"bank{d}")
                        inject(nb, d, s + 1, start=True)
                        bhhn_mms(nb, d, stop=False)
                        if s >= 1:
                            wh_mms(nb, d, hmap[d].rhs, stop=False)
                        nbanks[d] = nb
                # elementwise chain
                rzs, tmp, npre, nt, dmn = {}, {}, {}, {}, {}
                for d in (0, 1):
                    rzs[d] = wrk.tile([128, 4, NB], BF16, tag=f"rzs{d}",
                                      name=f"rzs{d}")
                    nc.scalar.activation(rzs[d][:], banks[d][:, 0:256],
                                         AF.Sigmoid)
                for d in (0, 1):
                    tmp[d] = wrk.tile([128, 2, NB], BF16, tag=f"tmp{d}",
                                      name=f"tmp{d}")
                    nc.vector.tensor_tensor(tmp[d][:], rzs[d][:, 0:2, :],
                                            banks[d][:, 256:384], OP.mult)
                    c = col(d, s)
                    npre[d] = wrk.tile([128, 2, NB], BF16, tag=f"npre{d}",
                                       name=f"npre{d}")
                    xn_src = (banks[d][:, 384:512] if x_mms is not None
                              else xT[d][:, 4:6, c * NB:(c + 1) * NB])
                    nc.vector.tensor_tensor(npre[d][:], tmp[d][:],
                                            xn_src, OP.add)
                for d in (0, 1):
                    nt[d] = wrk.tile([128, 2, NB], BF16, tag=f"nt{d}",
                                     name=f"nt{d}")
                    nc.scalar.activation(nt[d][:], npre[d][:], AF.Tanh)
                for d in (0, 1):
                    hc = zeros3[:] if s == 0 else hmap[d].full()
                    dmn[d] = wrk.tile([128, 2, NB], BF16, tag=f"dmn{d}",
                                      name=f"dmn{d}")
                    nc.vector.tensor_tensor(dmn[d][:], nt[d][:], hc, OP.subtract)
                for d in (0, 1):
                    g = wrk.tile([128, 2, NB], BF16, tag=f"g{d}", name=f"g{d}")
                    nc.vector.tensor_tensor(g[:], rzs[d][:, 2:4, :],
                                            dmn[d][:], OP.mult)
                    gt[d] = g
                for d in (0, 1):
                    hc = zeros3[:] if s == 0 else hmap[d].full()
                    hdst = make_hdst(d, s)
                    nc.gpsimd.tensor_tensor(hdst.full(), hc, gt[d][:], OP.add)
                    hmap[d] = hdst
                banks = nbanks
            return hmap

        # =================== phase 1: gru1 (direct x matmuls) ===========
        def x_mms1(bank, d, c, start):
            w = WIH1["1f" if d == 0 else "1b"]
            first = True
            for m in range(6):
                c0 = m * NB if m < 4 else 384 + (m - 4) * NB
                for ki in range(2):
                    nc.tensor.matmul(
                        bank[:, c0:c0 + NB],
                        w[ki][:, m * 128:(m + 1) * 128],
                        seqT[ki][:, c * NB:(c + 1) * NB],
                        start=(start and first), stop=False,
                        skip_group_check=not (start and first))
                    first = False

        gru_layer("1f", "1b", y1T, x_mms=x_mms1)

        # =================== phase 2: gru2 (batched x GEMM) =============
        def y1_rhs(ki, cb):
            return y1T[:, ki, cb * 512:(cb + 1) * 512]

        rev = list(range(sn // 512 - 1, -1, -1))
        ec = xgemm(xTf, WIH2["2f"], y1_rhs, EVB["2f"])
        ec = xgemm(xTb, WIH2["2b"], y1_rhs, EVB["2b"], ec, cb_order=rev)

        h2 = gru_layer("2f", "2b", None, xT=(xTf, xTb))

        # =================== decoder ====================================
        # dec_h = [h2f, h2b] @ w_adj.T + b_adj   (transposed: (2x128, 64))
        pd = pgem.tile([128, 512], F32, tag="pg", name="pd")
        first = True
        for m in range(2):
            for ki in range(4):
                rhs = h2[0].rhs(ki) if ki < 2 else h2[1].rhs(ki - 2)
                nc.tensor.matmul(pd[:, m * NB:(m + 1) * NB],
                                 WADJ[ki][:, m * 128:(m + 1) * 128], rhs,
                                 start=first, stop=(m == 1 and ki == 3),
                                 skip_group_check=not first)
                first = False
        hdec = wrk.tile([128, 2, NB], BF16, tag="hdec", name="hdec")
        for m in range(2):
            nc.scalar.activation(hdec[:, m, :], pd[:, m * NB:(m + 1) * NB],
                                 AF.Identity, bias=BADJC[:, m:m + 1])
        hdec_t = hdec

        inT = None
        for t in range(tl):
            bank = pstep.tile([128, 512], F32, tag="A0", name="bankd")
            # bias injects (opener: first brzd mm)
            for m in range(4):
                nc.tensor.matmul(bank[:, m * NB:(m + 1) * NB],
                                 BRZD[:, m * 128:(m + 1) * 128], ONES[:],
                                 start=(m == 0), stop=False,
                                 skip_group_check=(m != 0))
            for ch in range(2):
                nc.tensor.matmul(bank[:, 384 + ch * NB:384 + (ch + 1) * NB],
                                 BIHDN[:, ch * 128:(ch + 1) * 128], ONES[:],
                                 start=False, stop=False, skip_group_check=True)
                nc.tensor.matmul(bank[:, 256 + ch * NB:256 + (ch + 1) * NB],
                                 BHHDN[:, ch * 128:(ch + 1) * 128], ONES[:],
                                 start=False, stop=False, skip_group_check=True)
            # x-part (input is zero at t=0)
            if inT is not None:
                for m in range(6):
                    c0 = m * NB if m < 4 else 384 + (m - 4) * NB
                    nc.tensor.matmul(bank[:, c0:c0 + NB],
                                     WIHD[:, m * 128:(m + 1) * 128], inT[:],
                                     start=False, stop=False,
                                     skip_group_check=True)
            # h-part
            for m in range(6):
                c0 = m * NB if m < 4 else 256 + (m - 4) * NB
                for ki in range(2):
                    nc.tensor.matmul(bank[:, c0:c0 + NB],
                                     WHHD[ki][:, m * 128:(m + 1) * 128],
                                     hdec_t[:, ki, :],
                                     start=False,
                                     stop=(m == 5 and ki == 1),
                                     skip_group_check=True)
            rzs = wrk.tile([128, 4, NB], BF16, tag="rzsd", name="rzsd")
            nc.scalar.activation(rzs[:], bank[:, 0:256], AF.Sigmoid)
            tmp = wrk.tile([128, 2, NB], BF16, tag="tmpd", name="tmpd")
            nc.vector.tensor_tensor(tmp[:], rzs[:, 0:2, :], bank[:, 256:384],
                                    OP.mult)
            npre = wrk.tile([128, 2, NB], BF16, tag="npred", name="npred")
            nc.vector.tensor_tensor(npre[:], tmp[:], bank[:, 384:512], OP.add)
            nt = wrk.tile([128, 2, NB], BF16, tag="ntd", name="ntd")
            nc.scalar.activation(nt[:], npre[:], AF.Tanh)
            dmn = wrk.tile([128, 2, NB], BF16, tag="dmnd", name="dmnd")
            nc.vector.tensor_tensor(dmn[:], nt[:], hdec_t[:], OP.subtract)
            g = wrk.tile([128, 2, NB], BF16, tag="gd", name="gd")
            nc.vector.tensor_tensor(g[:], rzs[:, 2:4, :], dmn[:], OP.mult)
            hnew = wrk.tile([128, 2, NB], BF16, tag="hnd", name="hnd")
            nc.gpsimd.tensor_tensor(hnew[:], hdec_t[:], g[:], OP.add)
            hdec_t = hnew

            # fc1: outT = w_fc1 @ h' + b   -> (56, 64)
            pf = pgem.tile([128, 512], F32, tag="pg", name="pf")
            for ki in range(2):
                nc.tensor.matmul(pf[0:V, 0:NB], WFC1[ki][:, 0:V],
                                 hnew[:, ki, :],
                                 start=(ki == 0), stop=(ki == 1))
            outf = wrk.tile([V, NB], F32, tag="outf", name="outf")
            nc.scalar.activation(outf[:], pf[0:V, 0:NB], AF.Identity,
                                 bias=BFC1C[:, 0:1])
            nc.sync.dma_start(out=out_dram[t], in_=outf[:])
            if t + 1 < tl:
                it = wrk.tile([V, NB], BF16, tag="inT", name="inT")
                nc.vector.tensor_copy(it[:], outf[:])
                inT = it

        es.close()

    return nc


_PROG_CACHE = {}


def _get_program(nsteps, tl):
    key = (nsteps, tl)
    if key not in _PROG_CACHE:
        _PROG_CACHE[key] = build_program(nsteps, tl)
    return _PROG_CACHE[key]


def run_device(inputs, nsteps=NS, trace=False):
    tl = int(np.asarray(inputs["target_length"]))
    nc = _get_program(nsteps, tl)
    d = prepare_inputs(inputs, nsteps)
    res = run_bass_kernel_spmd(nc, [d], [0], trace=trace)
    out = res.results[0]["out"]          # [tl, V, NB]
    full = np.ascontiguousarray(
        np.transpose(out, (2, 0, 1)).astype(np.float32))
    return full, res


def kernel(**inputs):
    return run_device(inputs)[0]
